# revision 21
# baseline (speedup 1.0000x reference)
"""Trainium2 Bass kernel for AtomFeaturizer (embedding_lookup, 8 cores) — v2.

v1 built the one-hot features on-device (broadcast matmul + DVE is_equal);
the DVE is_equal (PSUM-f32 source, 1x mode) capped the pipeline at
~1.17ns/atom (~146us/core) and measured 191.6us end to end.

v2 moves the one-hot construction to the host and ships the features as
1-byte fp8_e4m3 rows, which eliminates the DVE is_equal AND the broadcast
matmul entirely. Feature rows per atom (K=86, all exact in fp8 except
scalar3):
  [0:75)   one-hot of the six categorical indices (46+6+11+3+5+4)
  [75:79)  bond-count histogram h[c] = #slots with count c, c=1..4 (0..4,
           exact in fp8). sum_c h_c*E_bond[c] == the masked bond-embedding
           sum, because the 4 slots share one table.
  [79:83)  bond_counts[:,j]/4 (exact: {0,.25,.5,.75,1})
  [83:86)  scalar3 (fp8 rounding ~2^-4 relative, scaled by |W|~0.1 -> noise)
The fused table T [86,128] bf16 carries the matching rows (b folded into
E_atom rows). One table-stationary matmul (bf16 stationary x fp8e4 moving,
1 col/cycle) per 512-column chunk produces out[128 dims, atoms] f32 in PSUM.

The output is quantized to int8 on-device: out_i8 = convert(psum *
(127/(1.02*A_d))) where A_d is a per-dim upper bound on |out| computed on
the host from the tiny tables (so no saturation can occur); the host
dequantizes. This halves the HBM write traffic vs bf16 (16MB vs 32MB per
core). Worst-case quantization error is one step = 1.02*A/127 ~ 0.8% of the
output scale even if the f32->int8 convert truncates — well inside the 2e-2
gate. The PSUM->SBUF cast is split between the DVE and ACT engines
(~0.45/0.55), the only remaining elementwise work.

Per-core steady state (125000 atoms): PE ~0.42ns/atom, DVE+ACT cast
~0.6ns/atom combined, DMA 86B in + 128B out = 214B/atom ~ 27MB -> ~75us at
HBM rate. Data parallel over atoms: 125000 per core, blocks of 5000 with a
SWDGE input DMA and alternating HWDGE output DMAs.

Measured (paired 64-pass differential, which reads ~25% above the
official-harness method judging by the v1 calibration): ~87us vs v1's 240.8
same-method (official 191.6). The binding constraint is the two-engine
PSUM->SBUF cast pipeline, NOT DMA: skipping the output DMA saves only
~5us, halving input DMA saves nothing, while the cast floor is
125k cols / (0.96+1.2 GHz) ~ 58us + per-group init. A/B-tested knobs
(kept=best): group 1000 w/ 4-deep psum rotation (2000/1250 w/ 2-deep lose
17-38us — rotation depth beats init amortization); dve_cols 448 (400/360/
512 lose 3-17us); output DMA alternating SP/ACT HWDGE queues (all-sync
-3us, sync+SWDGE -18us — SWDGE output stalls the Pool queue that feeds
input prefetch); block 5000 (25000 loses 34us); bufs_feat 7, bufs_outs 4,
hp 0 (+5us combined).

The fp8 rows travel as uint8 end-to-end (host packs raw e4m3 bytes; the
matmul moving operand bitcasts to float8e4) to avoid fp8 dtype handling in
the jax transfer path.
"""
import numpy as np
import ml_dtypes
from contextlib import ExitStack

from concourse import bacc, mybir
import concourse.bass as bass
import concourse.tile as tile
from concourse.bass_utils import run_bass_kernel_spmd

BF16 = ml_dtypes.bfloat16
FP8 = ml_dtypes.float8_e4m3
NCORES = 8
N_TOTAL = 1_000_000
D = 128

SEC_BASES = [0, 46, 52, 63, 66, 71]
SEC_SIZES = [46, 6, 11, 3, 5, 4]
K_MAIN = 86   # 75 one-hot + 4 hist + 4 counts/4 + 3 scalar3

GROUP = 1000   # atoms per group: one [128, group] f32 psum tile (2 banks)
HALF = 512     # max matmul output free size (one psum bank of f32)
BLOCK = 5000   # atoms per DMA block
N_CORE = 125000  # 1M atoms / 8 cores, no padding

_NC_CACHE = {}


def build_consts(E_atom, E_deg, E_chg, E_hyb, E_h, E_chi, E_bond, W, b):
    T = np.zeros((K_MAIN, D), np.float32)
    T[0:46] = E_atom + b[None, :]
    T[46:52] = E_deg
    T[52:63] = E_chg
    T[63:66] = E_hyb
    T[66:71] = E_h
    T[71:75] = E_chi
    T[75:79] = E_bond[1:5]
    T[79:83] = W[3:7]
    T[83:86] = W[0:3]

    # Per-dim upper bound on |out|: one row fires per categorical section,
    # each bond slot contributes at most max_c |E_bond[c]+(c/4)W[3+j]|, and
    # scalar3 in [0,1) contributes at most |W[k]|.
    A = np.zeros(D, np.float32)
    for base, size in zip(SEC_BASES, SEC_SIZES):
        A += np.abs(T[base:base + size]).max(axis=0)
    for j in range(4):
        slot = np.stack([np.abs(E_bond[c] + (c / 4.0) * W[3 + j])
                         for c in range(1, 5)])
        A += slot.max(axis=0)
    A += np.abs(W[0:3]).sum(axis=0)
    A = A * 1.02 + 1e-6   # margin for bf16/fp8 rounding in the matmul
    qvec = (127.0 / A).astype(np.float32)[:, None]      # [128,1] scale
    dq = (A / 127.0).astype(np.float32)                  # host dequant
    return T.astype(BF16), qvec, dq


def build_feat(atom_idx, degree_idx, charge_idx, hybrid_idx, numh_idx,
               chiral_idx, bond_counts, scalar3):
    n = atom_idx.shape[0]
    feat = np.zeros((K_MAIN, n), np.uint8)
    ar = np.arange(n)
    one = np.asarray(1.0, FP8).view(np.uint8)
    for base, idx in zip(SEC_BASES, [atom_idx, degree_idx, charge_idx,
                                     hybrid_idx, numh_idx, chiral_idx]):
        feat[base + idx, ar] = one
    cnt_lut = np.asarray([0, 1, 2, 3, 4], FP8).view(np.uint8)
    for c in range(1, 5):
        feat[75 + c - 1] = cnt_lut[(bond_counts == c).sum(axis=1)]
    q_lut = np.asarray([0.0, 0.25, 0.5, 0.75, 1.0], FP8).view(np.uint8)
    for j in range(4):
        feat[79 + j] = q_lut[bond_counts[:, j]]
    feat[83:86] = np.asarray(scalar3.T, dtype=FP8).view(np.uint8)
    return feat


def build_nc3(n_pad, block=BLOCK, bufs_feat=7, bufs_outs=4,
              bufs_ps=4, passes=1, out_piece=None, group=GROUP, out_eng=1,
              dve_cols=448, hp=0, cast_mode=0, pace=0, prescale=0,
              dbg_skip=""):
    if out_piece is None:
        out_piece = block
    key = ("v4", n_pad, block, bufs_feat, bufs_outs, bufs_ps, passes,
           out_piece, group, out_eng, dve_cols, hp, cast_mode, pace,
           prescale, dbg_skip)
    if key in _NC_CACHE:
        return _NC_CACHE[key]
    skip = set(dbg_skip.split(",")) if dbg_skip else set()
    assert n_pad % block == 0 and block % group == 0
    nblocks = n_pad // block
    ngroups = block // group
    chunks = []
    c0 = 0
    while c0 < group:
        c1 = min(c0 + HALF, group)
        chunks.append((c0, c1))
        c0 = c1
    bf = mybir.dt.bfloat16
    f32 = mybir.dt.float32
    u8 = mybir.dt.uint8
    i8 = mybir.dt.int8
    fp8 = mybir.dt.float8e4

    nc = bacc.Bacc("TRN2", target_bir_lowering=False, debug=False)
    feat_d = nc.dram_tensor("feat", [K_MAIN, n_pad], u8, kind="ExternalInput")
    t_d = nc.dram_tensor("t_mat", [K_MAIN, D], bf, kind="ExternalInput")
    qvec_d = nc.dram_tensor("qvec", [D, 1], f32, kind="ExternalInput")
    out_d = nc.dram_tensor("out", [D, n_pad], i8, kind="ExternalOutput")

    with tile.TileContext(nc) as tc, ExitStack() as ctx:
        consts = ctx.enter_context(tc.tile_pool(name="consts", bufs=1))
        feat_pool = ctx.enter_context(tc.tile_pool(name="feat", bufs=bufs_feat))
        outs_pool = ctx.enter_context(tc.tile_pool(name="outs", bufs=bufs_outs))
        ps_pool = ctx.enter_context(
            tc.tile_pool(name="ps", bufs=bufs_ps, space=bass.MemorySpace.PSUM))

        t_t = consts.tile([K_MAIN, D], bf)
        nc.sync.dma_start(t_t[:], t_d.ap())
        qvec_t = consts.tile([D, 1], f32)
        nc.sync.dma_start(qvec_t[:], qvec_d.ap())

        fap = feat_d.ap()
        oap = out_d.ap()

        total_blocks = nblocks * passes
        prefetch = bufs_feat - 1
        fetched = {}

        def fetch(i):
            if i >= total_blocks or i in fetched:
                return
            cols = slice((i % nblocks) * block, (i % nblocks + 1) * block)
            from contextlib import nullcontext
            with (tc.high_priority() if hp else nullcontext()):
                feat_t = feat_pool.tile([K_MAIN, block], u8)
                if "inhalf" in skip:   # timing probe: half the input bytes
                    nc.gpsimd.dma_start(feat_t[0:43, :], fap[0:43, cols])
                else:
                    nc.gpsimd.dma_start(feat_t[:], fap[:, cols])
            fetched[i] = feat_t

        for i in range(prefetch):
            fetch(i)
        for bi in range(total_blocks):
            blk = bi % nblocks
            fetch(bi)
            feat_t = fetched.pop(bi)
            outs_t = outs_pool.tile([D, block], i8)
            for g in range(ngroups):
                if pace:
                    # Pace the scheduler's internal sim at the true steady
                    # -state group period so the semaphore order it pins
                    # matches the real pipeline rhythm (its own cost model
                    # mis-rates the PE p-state and cast costs).
                    tc.tile_set_cur_wait((bi * ngroups + g) * pace * 1e-6)
                if g == 1:
                    fetch(bi + prefetch)
                lo = g * group
                ps = ps_pool.tile([D, group], f32)
                if "mm" not in skip:
                    spans = [(0, 1)] if "mm1" in skip else chunks
                    for h0, h1 in spans:
                        nc.tensor.matmul(
                            ps[:, h0:h1], t_t[:, :],
                            feat_t[0:K_MAIN, lo + h0:lo + h1].bitcast(fp8),
                            start=True, stop=True)
                # quantizing psum->int8 cast, split across DVE and ACT.
                # cast_mode 0: per-group column split (dve_cols to DVE).
                # cast_mode 1: whole-group alternation 2:3 (DVE g%5 in {0,3},
                # ACT the rest) — one instruction per group, half the inits.
                if "cast" not in skip:
                    if cast_mode == 1:
                        if g % 5 in (0, 3):
                            nc.vector.tensor_scalar(
                                outs_t[:, lo:lo + group], ps[:, 0:group],
                                qvec_t[:, 0:1], None, mybir.AluOpType.mult)
                        else:
                            nc.scalar.mul(outs_t[:, lo:lo + group],
                                          ps[:, 0:group], qvec_t[:, 0:1])
                    elif prescale:
                        # quantization scale folded into T on the host:
                        # the cast is a pure f32->int8 copy
                        dc = min(dve_cols, group)
                        if dc > 0:
                            nc.vector.tensor_copy(
                                outs_t[:, lo:lo + dc], ps[:, 0:dc])
                        if dc < group:
                            nc.scalar.copy(outs_t[:, lo + dc:lo + group],
                                           ps[:, dc:group])
                    else:
                        dc = min(dve_cols, group)
                        if dc > 0:
                            nc.vector.tensor_scalar(
                                outs_t[:, lo:lo + dc], ps[:, 0:dc],
                                qvec_t[:, 0:1], None, mybir.AluOpType.mult)
                        if dc < group:
                            nc.scalar.mul(outs_t[:, lo + dc:lo + group],
                                          ps[:, dc:group], qvec_t[:, 0:1])
            # output DMA per piece, alternating HWDGE queues; the final
            # block drains in per-group pieces so the last store overlaps
            # the tail of the compute pipeline
            if "out" not in skip:
                piece = group if blk == nblocks - 1 else out_piece
                for plo in range(0, block, piece):
                    cols = slice(blk * block + plo,
                                 blk * block + plo + piece)
                    piece_idx = blk + plo // piece
                    alt = {0: nc.sync, 1: nc.scalar, 2: nc.gpsimd}[out_eng]
                    eng = alt if piece_idx % 2 else nc.sync
                    eng.dma_start(oap[:, cols],
                                  outs_t[:, plo:plo + piece])
        assert not fetched
    nc.compile()
    _NC_CACHE[key] = nc
    return nc


def _prepare(inputs, block=BLOCK):
    inputs = {k: np.asarray(v) for k, v in inputs.items()}
    T, qvec, dq = build_consts(
        inputs['E_atom'].astype(np.float32), inputs['E_deg'].astype(np.float32),
        inputs['E_chg'].astype(np.float32), inputs['E_hyb'].astype(np.float32),
        inputs['E_h'].astype(np.float32), inputs['E_chi'].astype(np.float32),
        inputs['E_bond'].astype(np.float32), inputs['W'].astype(np.float32),
        inputs['b'].astype(np.float32))
    feat = build_feat(
        inputs['atom_idx'], inputs['degree_idx'], inputs['charge_idx'],
        inputs['hybrid_idx'], inputs['numh_idx'], inputs['chiral_idx'],
        inputs['bond_counts'], inputs['scalar3'])
    n = feat.shape[1]
    n_core = -(-n // NCORES)
    n_pad = -(-n_core // block) * block
    if n_pad * NCORES != n:
        pad = np.zeros((feat.shape[0], n_pad * NCORES - n), np.uint8)
        feat = np.concatenate([feat, pad], axis=1)
    in_maps = []
    for c in range(NCORES):
        f = feat[:, c * n_pad:(c + 1) * n_pad]
        in_maps.append({
            "feat": np.ascontiguousarray(f), "t_mat": T, "qvec": qvec,
        })
    return n, n_pad, in_maps, dq


def _run(inputs, trace=False, **kw):
    n_total, n_pad, in_maps, dq = _prepare(inputs)
    nc = build_nc3(n_pad)
    res = run_bass_kernel_spmd(nc, in_maps, list(range(NCORES)), trace=trace, **kw)
    out = np.concatenate(
        [res.results[c]["out"].T for c in range(NCORES)], axis=0)
    return out[:n_total].astype(np.float32) * dq[None, :], res


def kernel(**inputs) -> np.ndarray:
    out, _ = _run(inputs, trace=False)
    return out


# ---------------------------------------------------------------------------
# Timing harness (not used by kernel()): repeated on-device execution with
# pre-staged inputs and donated zero output buffers, mirroring
# bass2jax.run_bass_via_pjrt's shard_map build.
# ---------------------------------------------------------------------------

def _build_exec(nc, n_cores):
    import jax
    from jax.experimental.shard_map import shard_map
    from jax.sharding import Mesh, PartitionSpec
    from concourse import bass2jax

    bass2jax.install_neuronx_cc_hook()
    partition_name = (nc.partition_id_tensor.name
                      if nc.partition_id_tensor else None)
    in_names, out_names, out_avals = [], [], []
    for alloc in nc.m.functions[0].allocations:
        if not isinstance(alloc, mybir.MemoryLocationSet):
            continue
        name = alloc.memorylocations[0].name
        if alloc.kind == "ExternalInput":
            if name != partition_name:
                in_names.append(name)
        elif alloc.kind == "ExternalOutput":
            out_names.append(name)
            out_avals.append(jax.core.ShapedArray(
                tuple(alloc.tensor_shape), mybir.dt.np(alloc.dtype)))
    n_params = len(in_names)
    all_in = list(in_names + out_names)
    if partition_name is not None:
        all_in.append(partition_name)
    all_in = tuple(all_in)

    def _body(*args):
        operands = list(args)
        if partition_name is not None:
            operands.append(bass2jax.partition_id_tensor())
        outs = bass2jax._bass_exec_p.bind(
            *operands, out_avals=tuple(out_avals), in_names=all_in,
            out_names=tuple(out_names),
            lowering_input_output_aliases=(),
            sim_require_finite=True, sim_require_nnan=True, nc=nc)
        return tuple(outs)

    devices = jax.devices()[:n_cores]
    mesh = Mesh(np.asarray(devices), ("core",))
    nin = n_params + len(out_names)
    donate = tuple(range(n_params, nin))
    sharded = jax.jit(
        shard_map(_body, mesh=mesh, in_specs=(PartitionSpec("core"),) * nin,
                  out_specs=(PartitionSpec("core"),) * len(out_names),
                  check_rep=False),
        donate_argnums=donate, keep_unused=True)
    return sharded, mesh, in_names, out_names, out_avals


def time_nc(nc, in_maps, iters=16):
    import time as _time
    import jax
    from jax.sharding import NamedSharding, PartitionSpec

    sharded, mesh, in_names, out_names, out_avals = _build_exec(nc, NCORES)
    sh = NamedSharding(mesh, PartitionSpec("core"))
    gin = []
    for name in in_names:
        cat = np.concatenate([np.asarray(m[name]) for m in in_maps], axis=0)
        gin.append(jax.device_put(cat, sh))
    zero_sets = []
    for _ in range(iters + 1):
        zero_sets.append([
            jax.device_put(np.zeros((NCORES * av.shape[0], *av.shape[1:]),
                                    av.dtype), sh)
            for av in out_avals])
    r = sharded(*gin, *zero_sets[0])
    jax.block_until_ready(r)
    del r
    t0 = _time.perf_counter()
    rs = [sharded(*gin, *zero_sets[1 + i]) for i in range(iters)]
    jax.block_until_ready(rs)
    dt = _time.perf_counter() - t0
    return dt / iters * 1e9


def time_pair(nc_a, nc_b, in_maps_a, in_maps_b=None, reps=10):
    ta, tb = time_pair_raw(nc_a, nc_b, in_maps_a, in_maps_b, reps)
    ta, tb = sorted(ta), sorted(tb)
    return ta[len(ta) // 2] * 1e9, tb[len(tb) // 2] * 1e9


def time_pair_raw(nc_a, nc_b, in_maps_a, in_maps_b=None, reps=10):
    """Interleave executions of two kernels; return raw per-call second lists.

    Robust-ish to the multi-ms, drifting axon-relay dispatch overhead: the two
    kernels see the same overhead distribution, so median(b) - median(a)
    estimates the device-time difference."""
    import time as _time
    import jax
    from jax.sharding import NamedSharding, PartitionSpec

    if in_maps_b is None:
        in_maps_b = in_maps_a
    execs = []
    for nc, in_maps in ((nc_a, in_maps_a), (nc_b, in_maps_b)):
        sharded, mesh, in_names, out_names, out_avals = _build_exec(nc, NCORES)
        sh = NamedSharding(mesh, PartitionSpec("core"))
        gin = []
        for name in in_names:
            cat = np.concatenate([np.asarray(m[name]) for m in in_maps], axis=0)
            gin.append(jax.device_put(cat, sh))
        zeros = [
            jax.device_put(np.zeros((NCORES * av.shape[0], *av.shape[1:]),
                                    av.dtype), sh)
            for av in out_avals]
        execs.append((sharded, gin, zeros, out_avals, sh))

    def one_call(i):
        sharded, gin, zeros, out_avals, sh = execs[i]
        import jax as _jax
        t0 = _time.perf_counter()
        r = sharded(*gin, *zeros)
        _jax.block_until_ready(r)
        dt = _time.perf_counter() - t0
        # donation consumed the zero buffers; recycle outputs as next zeros
        execs[i] = (sharded, gin, list(r), out_avals, sh)
        return dt

    one_call(0), one_call(1)  # warmup/compile
    ta, tb = [], []
    for _ in range(reps):
        ta.append(one_call(0))
        tb.append(one_call(1))
    return ta, tb


def time_kernel(inputs, iters=16, **kw):
    n_core, n_pad, in_maps, dq = _prepare(inputs)
    nc = build_nc3(n_pad, **kw)
    return time_nc(nc, in_maps, iters)


# revision 23
# speedup vs baseline: 1.0718x; 1.0718x over previous
"""Trainium2 Bass kernel for AtomFeaturizer (embedding_lookup, 8 cores) — v2.

v1 built the one-hot features on-device (broadcast matmul + DVE is_equal);
the DVE is_equal (PSUM-f32 source, 1x mode) capped the pipeline at
~1.17ns/atom (~146us/core) and measured 191.6us end to end.

v2 moves the one-hot construction to the host and ships the features as
1-byte fp8_e4m3 rows, which eliminates the DVE is_equal AND the broadcast
matmul entirely. Feature rows per atom (K=86, all exact in fp8 except
scalar3):
  [0:75)   one-hot of the six categorical indices (46+6+11+3+5+4)
  [75:79)  bond-count histogram h[c] = #slots with count c, c=1..4 (0..4,
           exact in fp8). sum_c h_c*E_bond[c] == the masked bond-embedding
           sum, because the 4 slots share one table.
  [79:83)  bond_counts[:,j]/4 (exact: {0,.25,.5,.75,1})
  [83:86)  scalar3 (fp8 rounding ~2^-4 relative, scaled by |W|~0.1 -> noise)
The fused table T [86,128] bf16 carries the matching rows (b folded into
E_atom rows). One table-stationary matmul (bf16 stationary x fp8e4 moving,
1 col/cycle) per 512-column chunk produces out[128 dims, atoms] f32 in PSUM.

The output is quantized to int8 on-device: out_i8 = convert(psum *
(127/(1.02*A_d))) where A_d is a per-dim upper bound on |out| computed on
the host from the tiny tables (so no saturation can occur); the host
dequantizes. This halves the HBM write traffic vs bf16 (16MB vs 32MB per
core). Worst-case quantization error is one step = 1.02*A/127 ~ 0.8% of the
output scale even if the f32->int8 convert truncates — well inside the 2e-2
gate. The PSUM->SBUF cast is split between the DVE and ACT engines
(~0.45/0.55), the only remaining elementwise work.

Per-core steady state (125000 atoms): PE ~0.42ns/atom, DVE+ACT cast
~0.6ns/atom combined, DMA 86B in + 128B out = 214B/atom ~ 27MB -> ~75us at
HBM rate. Data parallel over atoms: 125000 per core, blocks of 5000 with a
SWDGE input DMA and alternating HWDGE output DMAs.

Measured (paired 64-pass differential, which reads ~25% above the
official-harness method judging by the v1 calibration): ~87us vs v1's 240.8
same-method (official 191.6). The binding constraint is the two-engine
PSUM->SBUF cast pipeline, NOT DMA: skipping the output DMA saves only
~5us, halving input DMA saves nothing, while the cast floor is
125k cols / (0.96+1.2 GHz) ~ 58us + per-group init. A/B-tested knobs
(kept=best): group 1000 w/ 4-deep psum rotation (2000/1250 w/ 2-deep lose
17-38us — rotation depth beats init amortization); dve_cols 448 (400/360/
512 lose 3-17us); output DMA alternating SP/ACT HWDGE queues (all-sync
-3us, sync+SWDGE -18us — SWDGE output stalls the Pool queue that feeds
input prefetch); block 5000 (25000 loses 34us); bufs_feat 7, bufs_outs 4,
hp 0 (+5us combined).

Round-2 probes (all kept OFF — each lost vs the baseline config): the
free-running cast pipeline alone (matmuls stubbed to 1 col via
dbg_skip='mm1') measures 77.5us, so the casts ARE the wall and pipeline
coupling costs only ~7us. cast_mode=1 (whole-group alternation, one cast
instr/group at 2:3 DVE:ACT) loses 19.5us — splitting each group across
both engines releases its psum tile ~2x sooner for the 4-deep rotation,
which dominates the saved init cycles. prescale=1 (127/A_d folded into T
columns, casts become pure tensor_copy/scalar.copy) loses 12.7us.
Scheduler pacing (tile_set_cur_wait at 0.6-0.75us/group) is a wash;
bufs_outs 6 -3us. Structural dead ends re-verified: no third cast engine
exists (GPSIMD has no PSUM port), DVE 2x modes need 16-bit or all-SBUF
operands, partition-splitting the cast doubles per-engine column work.

The fp8 rows travel as uint8 end-to-end (host packs raw e4m3 bytes; the
matmul moving operand bitcasts to float8e4) to avoid fp8 dtype handling in
the jax transfer path.
"""
import numpy as np
import ml_dtypes
from contextlib import ExitStack

from concourse import bacc, mybir
import concourse.bass as bass
import concourse.tile as tile
from concourse.bass_utils import run_bass_kernel_spmd

BF16 = ml_dtypes.bfloat16
FP8 = ml_dtypes.float8_e4m3
NCORES = 8
N_TOTAL = 1_000_000
D = 128

SEC_BASES = [0, 46, 52, 63, 66, 71]
SEC_SIZES = [46, 6, 11, 3, 5, 4]
K_MAIN = 86   # 75 one-hot + 4 hist + 4 counts/4 + 3 scalar3

GROUP = 1000   # atoms per group: one [128, group] f32 psum tile (2 banks)
HALF = 512     # max matmul output free size (one psum bank of f32)
BLOCK = 5000   # atoms per DMA block
N_CORE = 125000  # 1M atoms / 8 cores, no padding

_NC_CACHE = {}


def build_consts(E_atom, E_deg, E_chg, E_hyb, E_h, E_chi, E_bond, W, b):
    T = np.zeros((K_MAIN, D), np.float32)
    T[0:46] = E_atom + b[None, :]
    T[46:52] = E_deg
    T[52:63] = E_chg
    T[63:66] = E_hyb
    T[66:71] = E_h
    T[71:75] = E_chi
    T[75:79] = E_bond[1:5]
    T[79:83] = W[3:7]
    T[83:86] = W[0:3]

    # Per-dim upper bound on |out|: one row fires per categorical section,
    # each bond slot contributes at most max_c |E_bond[c]+(c/4)W[3+j]|, and
    # scalar3 in [0,1) contributes at most |W[k]|.
    A = np.zeros(D, np.float32)
    for base, size in zip(SEC_BASES, SEC_SIZES):
        A += np.abs(T[base:base + size]).max(axis=0)
    for j in range(4):
        slot = np.stack([np.abs(E_bond[c] + (c / 4.0) * W[3 + j])
                         for c in range(1, 5)])
        A += slot.max(axis=0)
    A += np.abs(W[0:3]).sum(axis=0)
    A = A * 1.02 + 1e-6   # margin for bf16/fp8 rounding in the matmul
    qvec = (127.0 / A).astype(np.float32)[:, None]      # [128,1] scale
    dq = (A / 127.0).astype(np.float32)                  # host dequant
    return T.astype(BF16), qvec, dq


def build_feat(atom_idx, degree_idx, charge_idx, hybrid_idx, numh_idx,
               chiral_idx, bond_counts, scalar3):
    n = atom_idx.shape[0]
    feat = np.zeros((K_MAIN, n), np.uint8)
    ar = np.arange(n)
    one = np.asarray(1.0, FP8).view(np.uint8)
    for base, idx in zip(SEC_BASES, [atom_idx, degree_idx, charge_idx,
                                     hybrid_idx, numh_idx, chiral_idx]):
        feat[base + idx, ar] = one
    cnt_lut = np.asarray([0, 1, 2, 3, 4], FP8).view(np.uint8)
    for c in range(1, 5):
        feat[75 + c - 1] = cnt_lut[(bond_counts == c).sum(axis=1)]
    q_lut = np.asarray([0.0, 0.25, 0.5, 0.75, 1.0], FP8).view(np.uint8)
    for j in range(4):
        feat[79 + j] = q_lut[bond_counts[:, j]]
    feat[83:86] = np.asarray(scalar3.T, dtype=FP8).view(np.uint8)
    return feat


def build_nc3(n_pad, block=BLOCK, bufs_feat=7, bufs_outs=4,
              bufs_ps=4, passes=1, out_piece=None, group=GROUP, out_eng=1,
              dve_cols=448, hp=0, cast_mode=0, pace=0, prescale=0,
              dbg_skip=""):
    if out_piece is None:
        out_piece = block
    key = ("v4", n_pad, block, bufs_feat, bufs_outs, bufs_ps, passes,
           out_piece, group, out_eng, dve_cols, hp, cast_mode, pace,
           prescale, dbg_skip)
    if key in _NC_CACHE:
        return _NC_CACHE[key]
    skip = set(dbg_skip.split(",")) if dbg_skip else set()
    assert n_pad % block == 0 and block % group == 0
    nblocks = n_pad // block
    ngroups = block // group
    chunks = []
    c0 = 0
    while c0 < group:
        c1 = min(c0 + HALF, group)
        chunks.append((c0, c1))
        c0 = c1
    bf = mybir.dt.bfloat16
    f32 = mybir.dt.float32
    u8 = mybir.dt.uint8
    i8 = mybir.dt.int8
    fp8 = mybir.dt.float8e4

    nc = bacc.Bacc("TRN2", target_bir_lowering=False, debug=False)
    feat_d = nc.dram_tensor("feat", [K_MAIN, n_pad], u8, kind="ExternalInput")
    t_d = nc.dram_tensor("t_mat", [K_MAIN, D], bf, kind="ExternalInput")
    qvec_d = nc.dram_tensor("qvec", [D, 1], f32, kind="ExternalInput")
    out_d = nc.dram_tensor("out", [D, n_pad], i8, kind="ExternalOutput")

    with tile.TileContext(nc) as tc, ExitStack() as ctx:
        consts = ctx.enter_context(tc.tile_pool(name="consts", bufs=1))
        feat_pool = ctx.enter_context(tc.tile_pool(name="feat", bufs=bufs_feat))
        outs_pool = ctx.enter_context(tc.tile_pool(name="outs", bufs=bufs_outs))
        ps_pool = ctx.enter_context(
            tc.tile_pool(name="ps", bufs=bufs_ps, space=bass.MemorySpace.PSUM))

        t_t = consts.tile([K_MAIN, D], bf)
        nc.sync.dma_start(t_t[:], t_d.ap())
        qvec_t = consts.tile([D, 1], f32)
        nc.sync.dma_start(qvec_t[:], qvec_d.ap())

        fap = feat_d.ap()
        oap = out_d.ap()

        total_blocks = nblocks * passes
        prefetch = bufs_feat - 1
        fetched = {}

        def fetch(i):
            if i >= total_blocks or i in fetched:
                return
            cols = slice((i % nblocks) * block, (i % nblocks + 1) * block)
            from contextlib import nullcontext
            with (tc.high_priority() if hp else nullcontext()):
                feat_t = feat_pool.tile([K_MAIN, block], u8)
                if "inhalf" in skip:   # timing probe: half the input bytes
                    nc.gpsimd.dma_start(feat_t[0:43, :], fap[0:43, cols])
                elif i % nblocks == 0:
                    # first block of a pass: land the first group's columns
                    # in a separate DMA so the pipeline starts ~1.3us sooner
                    # (tile hazards are region-level)
                    g0 = cols.start + group
                    nc.gpsimd.dma_start(feat_t[:, 0:group],
                                        fap[:, cols.start:g0])
                    nc.gpsimd.dma_start(feat_t[:, group:block],
                                        fap[:, g0:cols.stop])
                else:
                    nc.gpsimd.dma_start(feat_t[:], fap[:, cols])
            fetched[i] = feat_t

        for i in range(prefetch):
            fetch(i)
        for bi in range(total_blocks):
            blk = bi % nblocks
            fetch(bi)
            feat_t = fetched.pop(bi)
            outs_t = outs_pool.tile([D, block], i8)
            for g in range(ngroups):
                if pace:
                    # Pace the scheduler's internal sim at the true steady
                    # -state group period so the semaphore order it pins
                    # matches the real pipeline rhythm (its own cost model
                    # mis-rates the PE p-state and cast costs).
                    tc.tile_set_cur_wait((bi * ngroups + g) * pace * 1e-6)
                if g == 1:
                    fetch(bi + prefetch)
                lo = g * group
                ps = ps_pool.tile([D, group], f32)
                if "mm" not in skip:
                    spans = [(0, 1)] if "mm1" in skip else chunks
                    for h0, h1 in spans:
                        nc.tensor.matmul(
                            ps[:, h0:h1], t_t[:, :],
                            feat_t[0:K_MAIN, lo + h0:lo + h1].bitcast(fp8),
                            start=True, stop=True)
                # quantizing psum->int8 cast, split across DVE and ACT.
                # cast_mode 0: per-group column split (dve_cols to DVE).
                # cast_mode 1: whole-group alternation 2:3 (DVE g%5 in {0,3},
                # ACT the rest) — one instruction per group, half the inits.
                if "cast" not in skip:
                    if cast_mode == 1:
                        if g % 5 in (0, 3):
                            nc.vector.tensor_scalar(
                                outs_t[:, lo:lo + group], ps[:, 0:group],
                                qvec_t[:, 0:1], None, mybir.AluOpType.mult)
                        else:
                            nc.scalar.mul(outs_t[:, lo:lo + group],
                                          ps[:, 0:group], qvec_t[:, 0:1])
                    elif prescale:
                        # quantization scale folded into T on the host:
                        # the cast is a pure f32->int8 copy
                        dc = min(dve_cols, group)
                        if dc > 0:
                            nc.vector.tensor_copy(
                                outs_t[:, lo:lo + dc], ps[:, 0:dc])
                        if dc < group:
                            nc.scalar.copy(outs_t[:, lo + dc:lo + group],
                                           ps[:, dc:group])
                    else:
                        dc = min(dve_cols, group)
                        if dc > 0:
                            nc.vector.tensor_scalar(
                                outs_t[:, lo:lo + dc], ps[:, 0:dc],
                                qvec_t[:, 0:1], None, mybir.AluOpType.mult)
                        if dc < group:
                            nc.scalar.mul(outs_t[:, lo + dc:lo + group],
                                          ps[:, dc:group], qvec_t[:, 0:1])
            # output DMA per piece, alternating HWDGE queues; the final
            # block drains in per-group pieces so the last store overlaps
            # the tail of the compute pipeline
            if "out" not in skip:
                piece = group if blk == nblocks - 1 else out_piece
                for plo in range(0, block, piece):
                    cols = slice(blk * block + plo,
                                 blk * block + plo + piece)
                    piece_idx = blk + plo // piece
                    alt = {0: nc.sync, 1: nc.scalar, 2: nc.gpsimd}[out_eng]
                    eng = alt if piece_idx % 2 else nc.sync
                    eng.dma_start(oap[:, cols],
                                  outs_t[:, plo:plo + piece])
        assert not fetched
    nc.compile()
    _NC_CACHE[key] = nc
    return nc


def _prepare(inputs, block=BLOCK):
    inputs = {k: np.asarray(v) for k, v in inputs.items()}
    T, qvec, dq = build_consts(
        inputs['E_atom'].astype(np.float32), inputs['E_deg'].astype(np.float32),
        inputs['E_chg'].astype(np.float32), inputs['E_hyb'].astype(np.float32),
        inputs['E_h'].astype(np.float32), inputs['E_chi'].astype(np.float32),
        inputs['E_bond'].astype(np.float32), inputs['W'].astype(np.float32),
        inputs['b'].astype(np.float32))
    feat = build_feat(
        inputs['atom_idx'], inputs['degree_idx'], inputs['charge_idx'],
        inputs['hybrid_idx'], inputs['numh_idx'], inputs['chiral_idx'],
        inputs['bond_counts'], inputs['scalar3'])
    n = feat.shape[1]
    n_core = -(-n // NCORES)
    n_pad = -(-n_core // block) * block
    if n_pad * NCORES != n:
        pad = np.zeros((feat.shape[0], n_pad * NCORES - n), np.uint8)
        feat = np.concatenate([feat, pad], axis=1)
    in_maps = []
    for c in range(NCORES):
        f = feat[:, c * n_pad:(c + 1) * n_pad]
        in_maps.append({
            "feat": np.ascontiguousarray(f), "t_mat": T, "qvec": qvec,
        })
    return n, n_pad, in_maps, dq


def _run(inputs, trace=False, **kw):
    n_total, n_pad, in_maps, dq = _prepare(inputs)
    nc = build_nc3(n_pad)
    res = run_bass_kernel_spmd(nc, in_maps, list(range(NCORES)), trace=trace, **kw)
    out = np.concatenate(
        [res.results[c]["out"].T for c in range(NCORES)], axis=0)
    return out[:n_total].astype(np.float32) * dq[None, :], res


def kernel(**inputs) -> np.ndarray:
    out, _ = _run(inputs, trace=False)
    return out


# ---------------------------------------------------------------------------
# Timing harness (not used by kernel()): repeated on-device execution with
# pre-staged inputs and donated zero output buffers, mirroring
# bass2jax.run_bass_via_pjrt's shard_map build.
# ---------------------------------------------------------------------------

def _build_exec(nc, n_cores):
    import jax
    from jax.experimental.shard_map import shard_map
    from jax.sharding import Mesh, PartitionSpec
    from concourse import bass2jax

    bass2jax.install_neuronx_cc_hook()
    partition_name = (nc.partition_id_tensor.name
                      if nc.partition_id_tensor else None)
    in_names, out_names, out_avals = [], [], []
    for alloc in nc.m.functions[0].allocations:
        if not isinstance(alloc, mybir.MemoryLocationSet):
            continue
        name = alloc.memorylocations[0].name
        if alloc.kind == "ExternalInput":
            if name != partition_name:
                in_names.append(name)
        elif alloc.kind == "ExternalOutput":
            out_names.append(name)
            out_avals.append(jax.core.ShapedArray(
                tuple(alloc.tensor_shape), mybir.dt.np(alloc.dtype)))
    n_params = len(in_names)
    all_in = list(in_names + out_names)
    if partition_name is not None:
        all_in.append(partition_name)
    all_in = tuple(all_in)

    def _body(*args):
        operands = list(args)
        if partition_name is not None:
            operands.append(bass2jax.partition_id_tensor())
        outs = bass2jax._bass_exec_p.bind(
            *operands, out_avals=tuple(out_avals), in_names=all_in,
            out_names=tuple(out_names),
            lowering_input_output_aliases=(),
            sim_require_finite=True, sim_require_nnan=True, nc=nc)
        return tuple(outs)

    devices = jax.devices()[:n_cores]
    mesh = Mesh(np.asarray(devices), ("core",))
    nin = n_params + len(out_names)
    donate = tuple(range(n_params, nin))
    sharded = jax.jit(
        shard_map(_body, mesh=mesh, in_specs=(PartitionSpec("core"),) * nin,
                  out_specs=(PartitionSpec("core"),) * len(out_names),
                  check_rep=False),
        donate_argnums=donate, keep_unused=True)
    return sharded, mesh, in_names, out_names, out_avals


def time_nc(nc, in_maps, iters=16):
    import time as _time
    import jax
    from jax.sharding import NamedSharding, PartitionSpec

    sharded, mesh, in_names, out_names, out_avals = _build_exec(nc, NCORES)
    sh = NamedSharding(mesh, PartitionSpec("core"))
    gin = []
    for name in in_names:
        cat = np.concatenate([np.asarray(m[name]) for m in in_maps], axis=0)
        gin.append(jax.device_put(cat, sh))
    zero_sets = []
    for _ in range(iters + 1):
        zero_sets.append([
            jax.device_put(np.zeros((NCORES * av.shape[0], *av.shape[1:]),
                                    av.dtype), sh)
            for av in out_avals])
    r = sharded(*gin, *zero_sets[0])
    jax.block_until_ready(r)
    del r
    t0 = _time.perf_counter()
    rs = [sharded(*gin, *zero_sets[1 + i]) for i in range(iters)]
    jax.block_until_ready(rs)
    dt = _time.perf_counter() - t0
    return dt / iters * 1e9


def time_pair(nc_a, nc_b, in_maps_a, in_maps_b=None, reps=10):
    ta, tb = time_pair_raw(nc_a, nc_b, in_maps_a, in_maps_b, reps)
    ta, tb = sorted(ta), sorted(tb)
    return ta[len(ta) // 2] * 1e9, tb[len(tb) // 2] * 1e9


def time_pair_raw(nc_a, nc_b, in_maps_a, in_maps_b=None, reps=10):
    """Interleave executions of two kernels; return raw per-call second lists.

    Robust-ish to the multi-ms, drifting axon-relay dispatch overhead: the two
    kernels see the same overhead distribution, so median(b) - median(a)
    estimates the device-time difference."""
    import time as _time
    import jax
    from jax.sharding import NamedSharding, PartitionSpec

    if in_maps_b is None:
        in_maps_b = in_maps_a
    execs = []
    for nc, in_maps in ((nc_a, in_maps_a), (nc_b, in_maps_b)):
        sharded, mesh, in_names, out_names, out_avals = _build_exec(nc, NCORES)
        sh = NamedSharding(mesh, PartitionSpec("core"))
        gin = []
        for name in in_names:
            cat = np.concatenate([np.asarray(m[name]) for m in in_maps], axis=0)
            gin.append(jax.device_put(cat, sh))
        zeros = [
            jax.device_put(np.zeros((NCORES * av.shape[0], *av.shape[1:]),
                                    av.dtype), sh)
            for av in out_avals]
        execs.append((sharded, gin, zeros, out_avals, sh))

    def one_call(i):
        sharded, gin, zeros, out_avals, sh = execs[i]
        import jax as _jax
        t0 = _time.perf_counter()
        r = sharded(*gin, *zeros)
        _jax.block_until_ready(r)
        dt = _time.perf_counter() - t0
        # donation consumed the zero buffers; recycle outputs as next zeros
        execs[i] = (sharded, gin, list(r), out_avals, sh)
        return dt

    one_call(0), one_call(1)  # warmup/compile
    ta, tb = [], []
    for _ in range(reps):
        ta.append(one_call(0))
        tb.append(one_call(1))
    return ta, tb


def time_kernel(inputs, iters=16, **kw):
    n_core, n_pad, in_maps, dq = _prepare(inputs)
    nc = build_nc3(n_pad, **kw)
    return time_nc(nc, in_maps, iters)
